# revision 29
# baseline (speedup 1.0000x reference)
"""Trainium2 Bass kernel for nn_DisARM (point-proposal anchor weighting net).

Strategy (data-parallel, one batch element per NeuronCore, 8 cores):
  The whole network is a per-(point, anchor) MLP followed by a softmax +
  min-max normalization over the 64 anchors of each point.

  Per core, the 1024*64 = 65536 (point, anchor) columns stream through the
  TensorEngine in 512-column tiles.  All BatchNorm scales/biases and the
  feat_dis / spa_dis / concat / Wa1 layers are folded on the host into:
    - Wf1' (128->64, feeds tanh)                       [f1 matmul]
    - a single 124x57 block matrix S that simultaneously computes
        s1_pre  = Ws1' @ loc          (3 -> 8)
        s2_pre  = Ws2' @ s1           (8 -> 16)
        agg_pre = Wc_f @ h + Wc_s @ s2 (64+16 -> 32)
        out_pre = Wa2 @ a             (32 -> 1)
      in ONE matmul per column tile   [packed matmul]
  Wf1' lives in PE-array columns 0-63 and S in columns 64-120, so both
  stationary operands stay resident with no per-tile weight reloads.

  The f1 and packed matmuls are issued interleaved so adjacent pairs run
  concurrently in the PE array.

  Layer chaining is software-pipelined through a wide fp16 SBUF "staging"
  buffer [124 partitions x W_STG]: rows 0-119 receive tanh(psum) from a
  single fused ScalarE activation per 4-tile group (h, s1, s2, a and
  t=tanh(out) all at once, with per-partition folded biases), row 120 is t,
  rows 121-123 hold the transposed locations (DMA'd in per-group pieces —
  engine APs must start at partition 0/32/64/96, DMA APs are exempt).
  Each activation writes SK=8 tiles ahead of where its group's matmuls
  read, so ScalarE (the roofline engine) streams gap-free while TensorE
  works two groups ahead; pipeline depth is 4 stages x 8 = 32 tiles.

  The t row is progressively extracted to DRAM in 8-tile pieces (the t row
  lives on ONE SBUF partition, so each 8 KiB extract is port-limited to
  ~3.2us — extracts are scheduled >=2 groups before their consumers).  The
  finale (exp/sum/min/max/normalize) runs in three pieces so nearly all of
  it overlaps the act stream: A = points 0-511 (g23/g24), B = 512-767
  (g33/g34), C = 768-1023 gathered as [128 partitions x 128 free] in the
  epilogue (vector-op cost is free-size-bound, so the narrow layout makes
  the tail chain ~4x cheaper than a [32, 512] one).

  Engine-queue assignment: feature chunks, t-extracts, gathers and most
  outputs on sync (HWDGE rings fan out across the 16 SDMA engines, so a
  port-limited extract does not serialize the stream behind it); loc
  pieces on gpsimd (SWDGE — must carry nothing else, its queue serializes
  and its end-of-kernel drain is several us unless it goes idle early);
  the final wn output on the scalar queue so the two tail outputs config
  concurrently.
"""

import numpy as np

BZ, NUM, NA, FD = 8, 1024, 64, 128
BN_EPS = 1e-5
N = NUM * NA          # 65536 columns per core
T = 512               # columns per matmul tile
G = 4                 # tiles per activation group
SK = 2 * G            # stage skew in tiles: act of group g feeds group g+2,
                      # so ScalarE runs concurrently with TensorE
NT = N // T           # 128 data tiles
N_MM = NT + 3 * SK    # 140 matmul iterations (pipeline flush)
N_GRP = N_MM // G     # 70 groups
W_STG = T * (N_MM + SK)  # host-side loc padding width
CHUNK = 2048          # feature DMA chunk (4 tiles, 512 KiB fp16)

_CACHE = {}


def _build_bass():
    """Build the Bass/Tile graph (shapes are static; one graph for all cores)."""
    from contextlib import ExitStack

    import concourse.bacc as bacc
    import concourse.mybir as mybir
    import concourse.tile as tile

    f16 = mybir.dt.float16
    f32 = mybir.dt.float32
    Tanh = mybir.ActivationFunctionType.Tanh
    Exp = mybir.ActivationFunctionType.Exp
    AX = mybir.AxisListType.X

    nc = bacc.Bacc()

    featD = nc.dram_tensor("feat", [FD, N], f16, kind="ExternalInput")
    locD = nc.dram_tensor("loc", [3, W_STG], f16, kind="ExternalInput")
    wf1D = nc.dram_tensor("wf1t", [FD, 64], f16, kind="ExternalInput")
    smatD = nc.dram_tensor("smat", [124, 57], f16, kind="ExternalInput")
    biasD = nc.dram_tensor("biasv", [121, 1], f32, kind="ExternalInput")
    wOutD = nc.dram_tensor("w_out", [128, T], f32, kind="ExternalOutput")
    wnOutD = nc.dram_tensor("wn_out", [128, T], f32, kind="ExternalOutput")

    with ExitStack() as ctx:
        tc = ctx.enter_context(tile.TileContext(nc))
        const = ctx.enter_context(tc.tile_pool(name="const", bufs=1))
        stg_pool = ctx.enter_context(tc.tile_pool(name="stg", bufs=1))
        feat_pool = ctx.enter_context(tc.tile_pool(name="featp", bufs=9))
        psum_pool = ctx.enter_context(tc.tile_pool(name="ps", bufs=2, space="PSUM"))
        fin = ctx.enter_context(tc.tile_pool(name="fin", bufs=1))
        dram = ctx.enter_context(tc.tile_pool(name="dram", bufs=1, space="DRAM"))

        # Staging rows: 0-63 h | 64-71 s1 | 72-87 s2 | 88-119 a | 120 t |
        # 121-123 loc.  Engine accesses must start at partition 0/32/64/96,
        # so the activation dest is stg[0:121] (identity map from psum rows)
        # and the loc rows (DMA-only) live at the top.
        stg = stg_pool.tile([124, W_STG], f16, tag="stg")
        # Head: first SK tiles of layer rows are read before any act writes
        # them.  Split per 2 tiles so the group-0 packed matmuls gate only on
        # the first piece instead of the whole 4096-column memset.
        for k in range(SK // 2):
            nc.vector.memset(stg[0:121, k * 2 * T : (k + 1) * 2 * T], 0.0)

        # loc lands on only 3 SBUF partitions (~1 AXI port), so one big DMA
        # would hog 3 SDMA engines for ~18us and stall everything queued
        # behind it.  Split it into per-group pieces issued in consumption
        # order (loc is host-padded with zeros to W_STG for the flush iters).
        N_LOC = N_MM // G

        def loc_piece(p):
            sl = slice(p * G * T, (p + 1) * G * T)
            nc.gpsimd.dma_start(out=stg[121:124, sl], in_=locD[:, sl])

        feat_chunks = {}
        wf1 = const.tile([FD, 64], f16, tag="wf1")
        smat = const.tile([124, 57], f16, tag="smat")
        biasv = const.tile([121, 1], f32, tag="biasv")

        def chunk(c, eng=None):
            if c not in feat_chunks:
                t_ = feat_pool.tile([FD, CHUNK], f16, tag="featc")
                # All feature chunks on HWDGE (sync): one ring still fans out
                # across all 16 SDMA engines, and it avoids the SWDGE Q7
                # emission jitter; gpsimd carries only loc pieces + outputs.
                # The first chunks are split per tile so the first matmuls
                # start ASAP (subtile deps: each waits only on its slice).
                nsplit = CHUNK // T if c < 2 else 1
                step = CHUNK // nsplit
                for s in range(nsplit):
                    lo = c * CHUNK + s * step
                    (eng or nc.sync).dma_start(
                        out=t_[:, s * step : (s + 1) * step],
                        in_=featD[:, lo : lo + step],
                    )
                feat_chunks[c] = t_
            return feat_chunks[c]

        N_CHUNK = N // CHUNK
        # Sync-queue order is latency-critical: each DGE config occupies the
        # sync engine ~600ns, and group 0 needs only wf1 + feature subtile 0
        # (all group-0/1 f1 matmuls read ft=0), then smat for the packed
        # matmuls and biasv for act 0.
        t0 = feat_pool.tile([FD, CHUNK], f16, tag="featc")
        nc.sync.dma_start(out=t0[:, 0:T], in_=featD[:, 0:T])
        feat_chunks[0] = t0
        nc.sync.dma_start(out=wf1[:, :], in_=wf1D[:, :])
        nc.sync.dma_start(out=smat[:, :], in_=smatD[:, :])
        nc.sync.dma_start(out=biasv[:, :], in_=biasD[:, :])
        for s in range(1, CHUNK // T):
            nc.sync.dma_start(
                out=t0[:, s * T : (s + 1) * T], in_=featD[:, s * T : (s + 1) * T]
            )
        for p in range(min(5, N_LOC)):
            loc_piece(p)

        # Fill the chunk-buffer window up front so the early groups never
        # wait on feature supply while the stream ramps.  Chunks 2-5 go via
        # the gpsimd queue: the scheduler models DMA transfers serially per
        # queue, and a sync queue stacked 6 chunks deep makes it believe the
        # mid-ramp chunks land ~10us late - it then gates the consuming
        # matmuls on artificial semaphores, stalling the real act stream.
        chunk(1)
        for c in range(2, 6):
            chunk(c, eng=nc.gpsimd)
        chunk(6)

        # ---- finale: softmax over anchors + min-max normalization ----
        # Three pieces: A = partitions 0-63, B = 64-95 (both overlap the act
        # stream), C = t tiles 96-127 gathered as [128, 256-col] rows in the
        # epilogue.  t(d) for data tile d sits at stg[120, T*(d+4*SK) : ...].
        t_dram = dram.tile([1, N], f16, tag="td")
        wfull = fin.tile([128, T], f16, tag="wfull")
        e32 = fin.tile([128, T], f32, tag="e32")
        ssum = fin.tile([128, 8], f32, tag="ssum")
        rs = fin.tile([128, 8], f32, tag="rs")
        w32 = fin.tile([128, T], f32, tag="w32")
        mn = fin.tile([128, 8], f32, tag="mn")
        mx = fin.tile([128, 8], f32, tag="mx")
        dd = fin.tile([128, 8], f32, tag="dd")
        rk = fin.tile([128, 8], f32, tag="rk")
        # Pieces B and C: 32 t-tiles each as [128 partitions, 2 groups, 64].
        CW = 32 * T // 128  # 128 free columns per partition
        wfullB = fin.tile([128, CW], f16, tag="wfullB")
        e32b = fin.tile([128, CW], f32, tag="e32b")
        w32b = fin.tile([128, CW], f32, tag="w32b")
        ssb = fin.tile([128, 2], f32, tag="ssb")
        rsb = fin.tile([128, 2], f32, tag="rsb")
        mnb = fin.tile([128, 2], f32, tag="mnb")
        mxb = fin.tile([128, 2], f32, tag="mxb")
        ddb = fin.tile([128, 2], f32, tag="ddb")
        rkb = fin.tile([128, 2], f32, tag="rkb")
        wfullC = fin.tile([128, CW], f16, tag="wfullC")
        e32c = fin.tile([128, CW], f32, tag="e32c")
        w32c = fin.tile([128, CW], f32, tag="w32c")
        ssc = fin.tile([128, 2], f32, tag="ssc")
        rsc = fin.tile([128, 2], f32, tag="rsc")
        mnc = fin.tile([128, 2], f32, tag="mnc")
        mxc = fin.tile([128, 2], f32, tag="mxc")
        ddc = fin.tile([128, 2], f32, tag="ddc")
        rkc = fin.tile([128, 2], f32, tag="rkc")

        # t(d) sits at stg[120, T*(d+4*SK)].
        def t_extract(tlo, thi):
            nc.sync.dma_start(
                out=t_dram[:, tlo * T : thi * T],
                in_=stg[120:121, (tlo + 4 * SK) * T : (thi + 4 * SK) * T],
            )

        def wfull_gather(p0, p1):
            # wfull partition p holds t_dram cols [512p, 512(p+1)).
            nc.sync.dma_start(
                out=wfull[p0:p1, :],
                in_=t_dram[0, p0 * T : p1 * T].rearrange("(p f) -> p f", p=p1 - p0),
            )

        def narrow_piece(wfullX, e32X, w32X, ssX, rsX, mnX, mxX, ddX, rkX):
            # exp / softmax / min-max on a [128, 2 groups, 64] narrow layout:
            # every op's cost is its free size, so this chain is ~4x cheaper
            # than the same math on a [32, 512] slice.
            nc.scalar.activation(out=e32X[:, :], in_=wfullX[:, :], func=Exp)
            e3x = e32X[:, :].rearrange("p (g a) -> p g a", a=NA)
            nc.vector.reduce_sum(out=ssX[:, :], in_=e3x, axis=AX)
            nc.vector.reciprocal(out=rsX[:, :], in_=ssX[:, :])
            w3x = w32X[:, :].rearrange("p (g a) -> p g a", a=NA)
            nc.vector.tensor_mul(w3x, e3x, rsX[:, :].broadcast_to((128, 2, NA)))
            nc.vector.tensor_reduce(out=mnX, in_=w3x, axis=AX, op=mybir.AluOpType.min)
            nc.vector.tensor_reduce(out=mxX, in_=w3x, axis=AX, op=mybir.AluOpType.max)
            nc.vector.tensor_sub(ddX[:, :], mxX[:, :], mnX[:, :])
            nc.vector.tensor_scalar_add(ddX[:, :], ddX[:, :], 1e-6)
            nc.vector.reciprocal(rkX[:, :], ddX[:, :])
            nc.vector.tensor_scalar_mul(rkX[:, :], rkX[:, :], 1.0 + 1e-6)
            wn3x = e32X[:, :].rearrange("p (g a) -> p g a", a=NA)
            nc.vector.tensor_sub(wn3x, w3x, mnX[:, :].broadcast_to((128, 2, NA)))
            nc.vector.tensor_mul(wn3x, wn3x, rkX[:, :].broadcast_to((128, 2, NA)))

        def finale_compute(P0, P1):
            NP = P1 - P0
            nc.scalar.activation(out=e32[P0:P1, :], in_=wfull[P0:P1, :], func=Exp)
            e3 = e32[P0:P1, :].rearrange("p (g a) -> p g a", a=NA)
            nc.vector.reduce_sum(out=ssum[P0:P1, :], in_=e3, axis=AX)
            nc.vector.reciprocal(out=rs[P0:P1, :], in_=ssum[P0:P1, :])
            w3 = w32[P0:P1, :].rearrange("p (g a) -> p g a", a=NA)
            nc.vector.tensor_mul(w3, e3, rs[P0:P1, :].broadcast_to((NP, 8, NA)))
            nc.vector.tensor_reduce(
                out=mn[P0:P1, :], in_=w3, axis=AX, op=mybir.AluOpType.min
            )
            nc.vector.tensor_reduce(
                out=mx[P0:P1, :], in_=w3, axis=AX, op=mybir.AluOpType.max
            )
            nc.vector.tensor_sub(dd[P0:P1, :], mx[P0:P1, :], mn[P0:P1, :])
            nc.vector.tensor_scalar_add(dd[P0:P1, :], dd[P0:P1, :], 1e-6)
            nc.vector.reciprocal(rk[P0:P1, :], dd[P0:P1, :])
            nc.vector.tensor_scalar_mul(rk[P0:P1, :], rk[P0:P1, :], 1.0 + 1e-6)
            # e32 is dead once w3 is computed; reuse it for wn (saves SBUF)
            wn3 = e32[P0:P1, :].rearrange("p (g a) -> p g a", a=NA)
            nc.vector.tensor_sub(wn3, w3, mn[P0:P1, :].broadcast_to((NP, 8, NA)))
            nc.vector.tensor_mul(wn3, wn3, rk[P0:P1, :].broadcast_to((NP, 8, NA)))

        for g in range(N_GRP):
            # Prefetch: chunk c feeds group c+2; stay 3-4 groups ahead.
            chunk(min(g + 3, N_CHUNK - 1))
            chunk(min(g + 4, N_CHUNK - 1))
            if 5 <= g + 4 < N_LOC:
                loc_piece(g + 4)
            # The t row drains at ~1 B/cycle (single partition), so extracts
            # are spread out; but sync-queue traffic during the ramp (g<15)
            # costs act stalls (HWDGE sem-slot recycling), so nothing fires
            # until g15: 8-tile pieces catch up over g15-22 (tiles 0-63 feed
            # gather A at g23), then 4-tile pieces track the act stream
            # (act g-1 completes t tiles [4g-28, 4g-25)).
            if 15 <= g <= 22:
                t_extract(8 * (g - 15), 8 * (g - 15) + 8)
            elif 23 <= g <= 37:
                t_extract(4 * (g - 7), 4 * (g - 7) + 4)
            if g == 23:
                wfull_gather(0, 64)
            if g == 25:
                finale_compute(0, 64)
            if g == 31:
                # t tiles 64-95 are all extracted by now
                nc.sync.dma_start(
                    out=wfullB[:, :],
                    in_=t_dram[0, 64 * T : 96 * T].rearrange("(p f) -> p f", p=128),
                )
            if g == 32:
                narrow_piece(wfullB, e32b, w32b, ssb, rsb, mnb, mxb, ddb, rkb)
            ps = psum_pool.tile([128, G * T], f32, tag="ps")
            # Interleave f1 and packed matmuls: adjacent pairs target
            # disjoint PE column groups (0-63 vs 64-120) and execute
            # concurrently in the array, roughly halving the per-group PE
            # span.  Flush iterations (i-SK > NT-1) skip f1: the h rows they
            # feed belong to out-of-range lineage, and stale psum values are
            # finite, so the fused act can read them harmlessly.
            for k in range(G):
                i = g * G + k
                if i - SK <= NT - 1:
                    ft = max(i - SK, 0)
                    ck = chunk(ft // (CHUNK // T))
                    sl = ft % (CHUNK // T)
                    nc.tensor.matmul(
                        ps[0:64, k * T : (k + 1) * T],
                        wf1[:, :],
                        ck[:, sl * T : (sl + 1) * T],
                        start=True,
                        stop=True,
                    )
                nc.tensor.matmul(
                    ps[64:121, k * T : (k + 1) * T],
                    smat[:, :],
                    stg[0:124, i * T : (i + 1) * T],
                    start=True,
                    stop=True,
                )
            # One fused tanh for h/s1/s2/a/t of the whole group, written SK
            # tiles ahead of where this group's matmuls read.  The LAST group
            # is split into two half-width acts so the final (port-limited)
            # t extracts can start ~1us earlier.
            if g == N_GRP - 1:
                for h in range(2):
                    nc.scalar.activation(
                        out=stg[
                            0:121,
                            (g * G + SK + 2 * h) * T : (g * G + SK + 2 * h + 2) * T,
                        ],
                        in_=ps[0:121, 2 * h * T : (2 * h + 2) * T],
                        func=Tanh,
                        bias=biasv[:, 0:1],
                        scale=1.0,
                    )
            else:
                nc.scalar.activation(
                    out=stg[0:121, (g * G + SK) * T : (g * G + SK + G) * T],
                    in_=ps[0:121, :],
                    func=Tanh,
                    bias=biasv[:, 0:1],
                    scale=1.0,
                )

        # Epilogue: piece C in the narrow [128, CW] layout (vector ops are
        # free-size-bound, so this costs ~1/4 of a [32, 512] chain).
        # Partition p holds t cols [96*T + CW*p, ...) = 2 softmax groups.
        # Partitions 0-111 (t tiles 96-123, all extracted by act 36) gather
        # from the DRAM bounce immediately; partitions 112-127 (t tiles
        # 124-127) bounce through DRAM after the last act (an SBUF->SBUF
        # scatter from the single-partition t-row is an illegal SBUF AP).
        # The three piece-C DMAs go on the SCALAR queue: it is idle after the
        # last tanh, so their configs can never be blocked behind an output
        # DMA waiting on a DVE-chain semaphore (the scheduler orders each
        # engine queue by its own simulated ready time, not program order).
        nc.scalar.dma_start(
            out=t_dram[:, 124 * T : 126 * T],
            in_=stg[120:121, (124 + 4 * SK) * T : (126 + 4 * SK) * T],
        )
        nc.scalar.dma_start(
            out=t_dram[:, 126 * T : 128 * T],
            in_=stg[120:121, (126 + 4 * SK) * T : (128 + 4 * SK) * T],
        )
        nc.scalar.dma_start(
            out=wfullC[0:112, :],
            in_=t_dram[0, 96 * T : 124 * T].rearrange("(p f) -> p f", p=112),
        )
        nc.scalar.dma_start(
            out=wfullC[112:128, :],
            in_=t_dram[0, 124 * T : 128 * T].rearrange("(p f) -> p f", p=16),
        )
        narrow_piece(wfullC, e32c, w32c, ssc, rsc, mnc, mxc, ddc, rkc)
        # All output DMAs go dead last: anything that waits on a DVE-chain
        # semaphore blocks its queue, so nothing may sit behind an output.
        nc.sync.dma_start(
            out=wOutD[96:128, :].rearrange("q (p f) -> (q p) f", p=4),
            in_=w32c[:, :],
        )
        nc.scalar.dma_start(
            out=wnOutD[96:128, :].rearrange("q (p f) -> (q p) f", p=4),
            in_=e32c[:, :],
        )
        nc.sync.dma_start(
            out=wOutD[64:96, :].rearrange("q (p f) -> (q p) f", p=4),
            in_=w32b[:, :],
        )
        nc.scalar.dma_start(
            out=wnOutD[64:96, :].rearrange("q (p f) -> (q p) f", p=4),
            in_=e32b[:, :],
        )
        nc.sync.dma_start(out=wOutD[0:64, :], in_=w32[0:64, :])
        nc.sync.dma_start(out=wnOutD[0:64, :], in_=e32[0:64, :])

    nc.compile()
    return nc


def _fold_weights(inputs):
    """Fold BN + layer compositions into Wf1'/S/bias on the host (float64)."""
    W = {k: np.asarray(v, dtype=np.float64) for k, v in inputs.items()
         if k not in ("locations", "features")}
    sf1 = W["gf1"] / np.sqrt(1.0 + BN_EPS)
    W1 = W["Wf1"] * sf1[:, None]
    b1 = W["bf1"] * sf1 + W["btf1"]
    ss1 = W["gs1"] / np.sqrt(1.0 + BN_EPS)
    Ws1p = W["Ws1"] * ss1[:, None]
    bs1p = W["bs1"] * ss1 + W["bts1"]
    ss2 = W["gs2"] / np.sqrt(1.0 + BN_EPS)
    Ws2p = W["Ws2"] * ss2[:, None]
    bs2p = W["bs2"] * ss2 + W["bts2"]
    sa1 = W["ga1"] / np.sqrt(1.0 + BN_EPS)
    Wa1s, Wa1f = W["Wa1"][:, :32], W["Wa1"][:, 32:]
    Wc_s = sa1[:, None] * (Wa1s @ W["Ws3"])      # (32, 16)
    Wc_f = sa1[:, None] * (Wa1f @ W["Wf2"])      # (32, 64)
    bc = sa1 * (Wa1s @ W["bs3"] + Wa1f @ W["bf2"] + W["ba1"]) + W["bta1"]
    Wa2, ba2 = W["Wa2"], W["ba2"]

    # Block matrix S [124 K-rows, 57 M-cols]; staging rows:
    #   0-63 h | 64-71 s1 | 72-87 s2 | 88-119 a | 120 t | 121-123 loc
    # psum rows (packed matmul M at col offset 64):
    #   64-71 s1_pre | 72-87 s2_pre | 88-119 agg_pre | 120 out_pre
    S = np.zeros((124, 57), np.float64)
    S[0:64, 24:56] = Wc_f.T         # h -> agg_pre
    S[64:72, 8:24] = Ws2p.T         # s1 -> s2_pre
    S[72:88, 24:56] = Wc_s.T        # s2 -> agg_pre
    S[88:120, 56:57] = Wa2.T        # a -> out_pre
    S[121:124, 0:8] = Ws1p.T        # loc -> s1_pre

    bias = np.zeros((121, 1), np.float32)
    bias[0:64, 0] = b1
    bias[64:72, 0] = bs1p
    bias[72:88, 0] = bs2p
    bias[88:120, 0] = bc
    bias[120, 0] = ba2[0]

    return (
        np.ascontiguousarray(W1.T.astype(np.float16)),   # [128, 64] lhsT
        np.ascontiguousarray(S.astype(np.float16)),      # [124, 57] lhsT
        bias,                                            # [121, 1] f32
    )


def _ensure_axon_hooks_importable():
    """bass_utils imports antenv.axon_hooks when tracing is requested (e.g.
    via a stray BASS_TRACE env var); provide a null shim if it's missing so
    execution degrades to no-trace instead of crashing."""
    try:
        import antenv.axon_hooks  # noqa: F401
    except ImportError:
        import sys
        import types

        import antenv

        mod = types.ModuleType("antenv.axon_hooks")
        _state = {"h": None}
        mod.set_axon_ntff_profile_hook = lambda h: _state.__setitem__("h", h)
        mod.get_axon_ntff_profile_hook = lambda: _state["h"]
        sys.modules["antenv.axon_hooks"] = mod
        antenv.axon_hooks = mod


def _run(inputs, trace=False):
    _ensure_axon_hooks_importable()
    from concourse.bass_utils import run_bass_kernel_spmd

    if "nc" not in _CACHE:
        _CACHE["nc"] = _build_bass()
    nc = _CACHE["nc"]

    wf1t, smat, biasv = _fold_weights(inputs)
    features = np.asarray(inputs["features"], dtype=np.float32)
    locations = np.asarray(inputs["locations"], dtype=np.float32)

    in_maps = []
    for b in range(BZ):
        feat16 = np.ascontiguousarray(
            features[b].reshape(FD, N).astype(np.float16))
        loc16 = np.zeros((3, W_STG), np.float16)
        loc16[:, :N] = locations[b].transpose(2, 0, 1).reshape(3, N)
        in_maps.append({
            "feat": feat16,
            "loc": loc16,
            "wf1t": wf1t,
            "smat": smat,
            "biasv": biasv,
        })

    res = run_bass_kernel_spmd(nc, in_maps, core_ids=list(range(BZ)), trace=trace)

    w = np.zeros((BZ, 1, NUM, NA), np.float32)
    wn = np.zeros((BZ, 1, NUM, NA), np.float32)
    for b in range(BZ):
        w[b, 0] = res.results[b]["w_out"].reshape(NUM, NA)
        wn[b, 0] = res.results[b]["wn_out"].reshape(NUM, NA)
    return (w, wn), res


def kernel(**inputs):
    (w, wn), _ = _run(inputs, trace=False)
    return (w, wn)

